# revision 32
# baseline (speedup 1.0000x reference)
"""GAT message-passing kernel for TRN2 (8-core SPMD).

Math (heads h, nodes n):
  t[n,h,:] = x[n] @ Ws[h].T            (t-space features, 64 per head)
  Ar[n,h]  = x[n] @ war[:,h]           (war = Ws[h].T @ a_r[h], folded weights)
  u        = exp(Ar)
  out[i, h*64:h*64+64] = elu( sum_{e:src=i} u[dst,h]*t[dst,h,:] / sum u[dst,h] )

Sharding: src-range per core. Phase 1 builds the Y table
[u*t (512) | u (8) | pad] for all nodes (replicated on every core, lives in
core-private DRAM, split in two halves so gather indices fit int16).
Phase 2 gathers per-edge Y rows (edges sorted by (window, half), padded to
128-edge blocks), builds a one-hot S on DVE, and segment-sums via PE matmul
into PSUM per 128-node window; then normalize + elu + store.
"""

import math
import numpy as np
from contextlib import ExitStack

import concourse.bass as bass
import concourse.bacc as bacc
import concourse.mybir as mybir
from concourse.tile import TileContext
from concourse.tile import add_dep_helper

F32 = mybir.dt.float32
BF16 = mybir.dt.bfloat16
FP8 = mybir.dt.float8e4
I16 = mybir.dt.int16

P = 128
IN_FEAT = 256
HEADS = 8
OUT = 64
TD = HEADS * OUT  # 512
YW = 640          # row stride in elements (1280 B, multiple of 256)
YU = 520          # useful cols per row: 512 t~ + 8 u


class Config:
    def __init__(self, n_nodes, src, dst, n_cores=8, ch_max=None,
                 gather_cols=YW):
        self.n_cores = n_cores
        self.gather_cols = gather_cols

        self.n_nodes = n_nodes
        # nodes per core, multiple of 128
        self.w_per_core = math.ceil(n_nodes / (n_cores * P))
        self.npc = self.w_per_core * P
        self.n_pad = self.npc * n_cores          # padded node count (Y rows)
        self.x_tiles = self.n_pad // P
        # half split for int16 gather indices
        h0_tiles = self.x_tiles // 2
        assert h0_tiles * P < 32768 and (self.x_tiles - h0_tiles) * P < 32768
        self.h0_tiles = h0_tiles
        self.h0_rows = h0_tiles * P
        self.h1_rows = (self.x_tiles - h0_tiles) * P

        # ---- per-core edge grouping (by window, dst-half) ----
        W = self.w_per_core
        src = np.asarray(src, dtype=np.int64)
        dst = np.asarray(dst, dtype=np.int64)
        core = src // self.npc
        w = (src % self.npc) // P
        lsrc = src % P
        half = (dst >= self.h0_rows).astype(np.int64)
        lidx = dst - self.h0_rows * half

        counts = np.zeros((n_cores, W, 2), dtype=np.int64)
        np.add.at(counts, (core, w, half), 1)
        cap = counts.max(axis=0)  # [W, 2] max over cores
        self.cap_blocks = np.ceil(cap / P).astype(np.int64)  # C[w][h]
        self.tot_blocks = int(self.cap_blocks.sum())
        self.tot_idx = self.tot_blocks * P
        if ch_max is None:
            ch_max = 9   # 2 chunks/group: ~1024-1152-row calls (ring-sized)
        self.ch_max = ch_max

        # order edges: by (core, w, half), then by dst row within each group —
        # sorted rows give the gather DMA quasi-sequential HBM access
        order = np.lexsort((lidx, half, w, core))
        s_core, s_w, s_half = core[order], w[order], half[order]
        s_lsrc, s_lidx = lsrc[order], lidx[order]

        # block offsets per (w, half) in the packed stream (same per core)
        blk_off = np.zeros((W, 2), dtype=np.int64)
        acc = 0
        for wi in range(W):
            for hi in range(2):
                blk_off[wi, hi] = acc
                acc += self.cap_blocks[wi, hi]
        self.blk_off = blk_off

        # rank within each (core,w,half) group: groups are contiguous after sort
        gkey = (s_core * W + s_w) * 2 + s_half
        change = np.r_[True, gkey[1:] != gkey[:-1]]
        grp_start = np.flatnonzero(change)
        grp_id = np.cumsum(change) - 1
        grp_rank = np.arange(len(order)) - grp_start[grp_id]
        slot = blk_off[s_w, s_half] * P + grp_rank  # global slot within core stream

        # call table: one gather call per (w, half, chunk of <=ch_max blocks)
        calls = []
        for wi in range(W):
            for hi in range(2):
                c = int(self.cap_blocks[wi, hi])
                b0 = int(blk_off[wi, hi])
                nch = -(-c // ch_max)
                off = 0
                for ci in range(nch):
                    nb = c // nch + (1 if ci < c % nch else 0)
                    calls.append((wi, hi, b0 + off, nb))
                    off += nb
                assert off == c
        self.calls = calls

        # pack idx into wrapped [16, tot_idx/16] (call-granular): within a call
        # starting at slot g0 (mult of 128), element i -> [i%16, g0//16 + i//16]
        self.idx_packed = np.zeros((n_cores, 128, self.tot_idx // 16), np.int16)
        self.meta_packed = np.full((n_cores, P, self.tot_blocks), -1.0, np.float32)
        call_starts = np.array([b0 * P for (_, _, b0, nb) in calls], dtype=np.int64)
        call_of_slot_idx = np.searchsorted(call_starts, slot, side="right") - 1
        g0 = call_starts[call_of_slot_idx]
        i_in_call = slot - g0
        row16 = i_in_call % 16
        col16 = g0 // 16 + i_in_call // 16
        self.idx_packed[s_core, row16, col16] = s_lidx.astype(np.int16)
        # HW: each of the 8 GpSimd cores reads indices from its own
        # 16-partition group -> replicate the 16-row pattern across all 128.
        self.idx_packed[:, 16:, :] = np.tile(self.idx_packed[:, :16, :], (1, 7, 1))
        blk = slot // P
        pslot = slot % P
        self.meta_packed[s_core, pslot, blk] = s_lsrc.astype(np.float32)

        self.max_group = int(cap.max())
        self.pad_frac = (self.tot_idx * n_cores) / max(1, len(src)) - 1.0


def build_program(cfg: Config):
    nc = bacc.Bacc("TRN2", target_bir_lowering=False, debug=False,
                   num_devices=cfg.n_cores, num_swdge_queues=4)
    W = cfg.w_per_core
    GC = cfg.gather_cols

    # x^T, cast to bf16 on host: [IN_FEAT, n_pad]
    xt_d = nc.dram_tensor("xt", [cfg.x_tiles // 4 * P, 1024], BF16,
                          kind="ExternalInput")
    wcat_d = nc.dram_tensor("wcat", [IN_FEAT, TD], BF16, kind="ExternalInput")
    war_d = nc.dram_tensor("war", [IN_FEAT, HEADS], BF16, kind="ExternalInput")
    iota_d = nc.dram_tensor("iota", [P, P], BF16, kind="ExternalInput")
    idx_d = nc.dram_tensor("idx", [128, cfg.tot_idx // 16], I16, kind="ExternalInput")
    meta_d = nc.dram_tensor("meta", [P, cfg.tot_blocks], BF16,
                            kind="ExternalInput")
    out_d = nc.dram_tensor("out", [cfg.npc, TD], F32, kind="ExternalOutput")
    y0_d = nc.dram_tensor("y0", [cfg.h0_rows, YW], BF16, kind="Internal")
    y1_d = nc.dram_tensor("y1", [cfg.h1_rows, YW], BF16, kind="Internal")

    y_writes = [[], []]  # per half
    with TileContext(nc) as tc:
        with ExitStack() as ctx:
            # ---------------- consts (loaded first so gathers can start
            # as soon as their Y half is written) ----------------
            consts = ctx.enter_context(tc.tile_pool(name="consts", bufs=1))
            idx_sb = consts.tile([128, cfg.tot_idx // 16], I16, tag="idx")
            nc.sync.dma_start(idx_sb[:, :], idx_d[:, :])
            meta_sb = consts.tile([P, cfg.tot_blocks], BF16, tag="meta")
            nc.sync.dma_start(meta_sb[:, :], meta_d[:, :])
            iota = consts.tile([P, P], BF16, tag="iota")
            nc.sync.dma_start(iota[:, :], iota_d[:, :])
            neg1 = consts.tile([P, 1], F32, tag="neg1")
            nc.vector.memset(neg1[:, :], -1.0)
            wc = consts.tile([P, 2, TD], BF16, tag="wc")
            nc.sync.dma_start(wc[:, :, :], wcat_d.rearrange("(c p) n -> p c n", p=P))
            wr = consts.tile([P, 2, HEADS], BF16, tag="wr")
            nc.sync.dma_start(wr[:, :, :], war_d.rearrange("(c p) n -> p c n", p=P))

            # phase-2 pools allocated BEFORE phase 1: disjoint SBUF regions,
            # so early h0 gathers need not wait for phase-1 space to free.
            gpool = ctx.enter_context(tc.tile_pool(name="gath", bufs=9))
            spool = ctx.enter_context(tc.tile_pool(name="onehot", bufs=6))
            opool = ctx.enter_context(tc.tile_pool(name="outp", bufs=2))

            # ---------------- phase 1: build Y table ----------------
            # Tiles are processed in batches of B4 per DMA call: the sync
            # sequencer costs ~1.3us per dma_start, so per-tile DMAs make
            # phase 1 dispatch-bound, not bandwidth-bound.
            B4 = 4
            assert cfg.h0_tiles % B4 == 0 and cfg.x_tiles % B4 == 0
            with ExitStack() as p1:
                xin = p1.enter_context(tc.tile_pool(name="xin", bufs=4))
                yout = p1.enter_context(tc.tile_pool(name="yout", bufs=3))
                ps_t = p1.enter_context(tc.tile_pool(name="ps_t", bufs=4, space="PSUM"))
                ps_ar = p1.enter_context(tc.tile_pool(name="ps_ar", bufs=2, space="PSUM"))

                y0_v = y0_d.rearrange("(c p) w -> p c w", p=P)
                y1_v = y1_d.rearrange("(c p) w -> p c w", p=P)
                for t4 in range(cfg.x_tiles // B4):
                    xT = xin.tile([P, 2, B4 * P], BF16)
                    nc.sync.dma_start(
                        xT[:, :, :],
                        xt_d[t4 * P:(t4 + 1) * P, :].rearrange(
                            "p (c n) -> p c n", c=2))
                    ysb = yout.tile([P, B4, YU], BF16)
                    for k in range(B4):
                        pt = ps_t.tile([P, TD], F32, tag="pt")
                        par = ps_ar.tile([P, HEADS], F32, tag="par")
                        xk = xT[:, :, k * P:(k + 1) * P]
                        nc.tensor.matmul(par[:, :], xk[:, 0, :], wr[:, 0, :], start=True, stop=False)
                        nc.tensor.matmul(par[:, :], xk[:, 1, :], wr[:, 1, :], start=False, stop=True)
                        nc.tensor.matmul(pt[:, :], xk[:, 0, :], wc[:, 0, :], start=True, stop=False)
                        nc.tensor.matmul(pt[:, :], xk[:, 1, :], wc[:, 1, :], start=False, stop=True)
                        # Row layout: [t~ h0-7 (512) | u h0-7 (8)] contiguous.
                        # u = exp(Ar) into cols 512:520
                        nc.scalar.activation(
                            ysb[:, k, TD:YU], par[:, :],
                            mybir.ActivationFunctionType.Exp)
                        # t~ = t * u (broadcast u over the 64 dims of each head)
                        nc.vector.tensor_tensor(
                            ysb[:, k, 0:TD].rearrange("p (h o) -> p h o", h=HEADS),
                            pt[:, :].rearrange("p (h o) -> p h o", h=HEADS),
                            ysb[:, k, TD:YU].unsqueeze(2).broadcast_to([P, HEADS, OUT]),
                            mybir.AluOpType.mult,
                        )
                    # one contiguous 520-col (1040 B) write per row, B4 tiles
                    t = t4 * B4
                    if t < cfg.h0_tiles:
                        dst = y0_v[:, t:t + B4, 0:YU]
                    else:
                        tt = t - cfg.h0_tiles
                        dst = y1_v[:, tt:tt + B4, 0:YU]
                    hf = int(t >= cfg.h0_tiles)
                    wi_ = nc.sync.dma_start(dst, ysb[:, :, :])
                    y_writes[hf].append(wi_)

            # ---------------- phase 2: gather + segment sums ----------------
            ps_num = ctx.enter_context(tc.tile_pool(name="ps_num", bufs=4, space="PSUM"))
            ps_den = ctx.enter_context(tc.tile_pool(name="ps_den", bufs=4, space="PSUM"))

            fence_pending = [True, True]  # per half
            qn = [0]

            # group calls by window
            calls_by_w = [[] for _ in range(W)]
            for (wi, hi, b0, nb) in cfg.calls:
                calls_by_w[wi].append((hi, b0, nb))

            PRE = 3
            nblk = {}
            bi_ct = {}
            pn_t = {}
            pd_t = {}
            for wi in range(W):
                nblk[wi] = sum(nb for (_, _, nb) in calls_by_w[wi])
                bi_ct[wi] = 0

            def emit_half(wi, want_half):
                for (hi, b0, nb) in calls_by_w[wi]:
                    if hi != want_half:
                        continue
                    g = gpool.tile([P, cfg.ch_max, YW], BF16)
                    src_t = y0_d if hi == 0 else y1_d
                    g_inst = nc.gpsimd.dma_gather(
                        out_ap=g[:, 0:nb, :],
                        in_ap=src_t[:, :],
                        idxs_ap=idx_sb[:, b0 * 8:(b0 + nb) * 8],
                        num_idxs=nb * P,
                        num_idxs_reg=nb * P,
                        elem_size=GC,
                        elem_step=YW,
                        single_packet=(nb * P <= 1024),
                        queue_num=qn[0],
                    )
                    qn[0] = (qn[0] + 1) % 4
                    if fence_pending[hi]:
                        # phase fence: the gather's indexed DRAM read of the Y
                        # tables is invisible to Tile's dependency tracking;
                        # gathers run in order on GpSimd, so gating the first
                        # gather per half on that half's writes fences it.
                        for wr_ in y_writes[hi]:
                            add_dep_helper(g_inst.ins, wr_.ins,
                                           reason="gather reads Y table")
                        fence_pending[hi] = False
                    s = spool.tile([P, cfg.ch_max, P], FP8)
                    nc.vector.tensor_tensor(
                        s[:, 0:nb, :],
                        meta_sb[:, b0:b0 + nb].unsqueeze(2).broadcast_to([P, nb, P]),
                        iota[:, :].unsqueeze(1).broadcast_to([P, nb, P]),
                        mybir.AluOpType.is_equal,
                    )
                    pn, pd = pn_t[wi], pd_t[wi]
                    for j in range(nb):
                        st = (bi_ct[wi] == 0)
                        sp = (bi_ct[wi] == nblk[wi] - 1)
                        nc.tensor.matmul(pn[:, :], s[:, j, :],
                                         g[:, j, 0:TD],
                                         start=st, stop=sp, skip_group_check=True)
                        nc.tensor.matmul(pd[:, :], s[:, j, :],
                                         g[:, j, TD:YU],
                                         start=st, stop=sp, skip_group_check=True)
                        bi_ct[wi] += 1

            # prologue: h0 gathers of the first PRE windows run ahead of the
            # y1 fence (which blocks the Pool FIFO until phase 1 completes)
            for wi in range(min(PRE, W)):
                pn_new = ps_num.tile([P, TD], F32, tag="pn")
                pd_new = ps_den.tile([P, HEADS], F32, tag="pd")
                pn_t[wi], pd_t[wi] = pn_new, pd_new
                emit_half(wi, 0)
            for wi in range(W):
                emit_half(wi, 1)
                pn, pd = pn_t[wi], pd_t[wi]
                # ---- evict window ----
                den = opool.tile([P, HEADS], F32, tag="den")
                nc.vector.tensor_scalar_add(den[:, :], pd[:, :], 1e-30)
                rden = opool.tile([P, HEADS], F32, tag="rden")
                nc.vector.reciprocal(rden[:, :], den[:, :])
                hout = opool.tile([P, TD], F32, tag="hout")
                nc.vector.tensor_tensor(
                    hout[:, :].rearrange("p (h o) -> p h o", h=HEADS),
                    pn[:, :].rearrange("p (h o) -> p h o", h=HEADS),
                    rden[:, :].unsqueeze(2).broadcast_to([P, HEADS, OUT]),
                    mybir.AluOpType.mult,
                )
                # elu(z) = max(z,0) + exp(min(z,0)) - 1
                xm = opool.tile([P, TD], F32, tag="xm")
                nc.scalar.activation(xm[:, :], hout[:, :],
                                     mybir.ActivationFunctionType.Relu,
                                     scale=-1.0)
                ex = opool.tile([P, TD], F32, tag="ex")
                nc.scalar.activation(ex[:, :], xm[:, :],
                                     mybir.ActivationFunctionType.Exp,
                                     scale=-1.0)
                fin = opool.tile([P, TD], F32, tag="fin")
                nc.vector.scalar_tensor_tensor(
                    out=fin[:, :], in0=hout[:, :], scalar=0.0, in1=ex[:, :],
                    op0=mybir.AluOpType.max, op1=mybir.AluOpType.add,
                )
                fin2 = opool.tile([P, TD], F32, tag="fin2")
                nc.scalar.activation(fin2[:, :], fin[:, :],
                                     mybir.ActivationFunctionType.Identity,
                                     bias=neg1[:, :])
                nc.sync.dma_start(out_d[wi * P:(wi + 1) * P, :], fin2[:, :])
                nxt = wi + PRE
                if nxt < W:
                    pn_new = ps_num.tile([P, TD], F32, tag="pn")
                    pd_new = ps_den.tile([P, HEADS], F32, tag="pd")
                    pn_t[nxt], pd_t[nxt] = pn_new, pd_new
                    emit_half(nxt, 0)

    nc.compile()
    return nc


def host_prep(cfg: Config, x, Ws, As):
    import ml_dtypes as _md
    x = np.asarray(x, np.float32)
    Ws = np.asarray(Ws, np.float32)
    As = np.asarray(As, np.float32)
    n = x.shape[0]
    xt = np.zeros((IN_FEAT, cfg.n_pad), np.float32)
    xt[:, :n] = x.T
    nb4 = cfg.x_tiles // 4
    xtb = np.zeros((nb4, P, 1024), np.float32)
    for c in range(2):
        v = xt[c * P:(c + 1) * P, :].reshape(P, nb4, 512)
        xtb[:, :, c * 512:(c + 1) * 512] = v.transpose(1, 0, 2)
    xt = np.ascontiguousarray(xtb.reshape(nb4 * P, 1024)).astype(_md.bfloat16)
    # wcat[f, h*64+o] = Ws[h,o,f]
    wcat = Ws.transpose(2, 0, 1).reshape(IN_FEAT, TD).astype(_md.bfloat16)
    a_r = As[:, OUT:, 0]  # [H, O]
    war = np.einsum("hof,ho->fh", Ws, a_r).astype(_md.bfloat16)
    iota = np.tile(np.arange(P, dtype=np.float32), (P, 1)).astype(_md.bfloat16)
    meta = cfg.meta_packed.astype(_md.bfloat16)
    in_maps = []
    for c in range(cfg.n_cores):
        in_maps.append({
            "xt": xt, "wcat": wcat, "war": war,
            "iota": np.ascontiguousarray(iota),
            "idx": np.ascontiguousarray(cfg.idx_packed[c]),
            "meta": np.ascontiguousarray(meta[c]),
        })
    return in_maps


from concourse.bass_utils import run_bass_kernel_spmd

LAST_EXEC_TIME_NS = None


def kernel(x, src, dst, Ws, As):
    """Full-input entry point: shards internally across 8 NeuronCores."""
    global LAST_EXEC_TIME_NS
    x = np.asarray(x, np.float32)
    src = np.asarray(src)
    dst = np.asarray(dst)
    Ws = np.asarray(Ws, np.float32)
    As = np.asarray(As, np.float32)
    n = x.shape[0]

    cfg = Config(n, src, dst, n_cores=8)
    nc = build_program(cfg)
    in_maps = host_prep(cfg, x, Ws, As)
    import os as _os
    _trace = _os.environ.get("KERNEL_TRACE", "0") == "1"
    res = run_bass_kernel_spmd(nc, in_maps, core_ids=list(range(cfg.n_cores)),
                               trace=_trace)
    LAST_EXEC_TIME_NS = res.exec_time_ns
    out = np.concatenate([res.results[c]["out"] for c in range(cfg.n_cores)],
                         axis=0)[:n]
    return np.ascontiguousarray(out, dtype=np.float32)
